# revision 16
# baseline (speedup 1.0000x reference)
"""MoE audio projector kernel for 8 Trainium2 NeuronCores.

Strategy (expert-parallel, sparse dispatch):
  Host: depthwise conv + residual, fold K frames, RMSNorm, sigmoid router,
        top-2 + combine weights, per-expert token gather (all tiny FLOPs),
        and the final combine-weight scaling of expert outputs.
  Device (8 cores): core c handles expert c//2 with H-half c%2 over only the
        tokens routed to that expert, plus a 1/8 H-slice of the shared
        expert over all tokens. bf16 matmuls, fp32 PSUM accumulation.
  Host: sum shared partials, scatter-add combine-scaled expert partials.

Schedule: phase A streams all tokens once (both 512-token halves per
k-tile, 4 PSUM banks) for shared mm1 while every token tile stays
resident in SBUF; phase B runs expert mm1 straight out of those resident
tiles (no copies); phase C does expert mm2 with O on partitions (exact
`cnt` columns, no 128-padding waste); phase D is shared mm2. Expert
outputs are written unscaled in O-major layout; the host applies the
combine weights.

DMA uses TWO queues that overlap transfers: the sync (SP) queue carries
the ci=0 token half + shared weights + expert mm1 weights; the gpsimd
(Pool) queue carries the ci=1 token half + biases + mm2 weights +
expert-output writes. Each queue is FIFO, so transfers are emitted in
consumption order and gated loads never sit ahead of urgent ones.
PSUM->SBUF copies are split across DVE and the scalar engine so neither
serializes the mm2 phases.
"""

import math

import numpy as np
import ml_dtypes

import concourse.bass as bass
import concourse.bacc as bacc
import concourse.mybir as mybir
import concourse.tile as tile
from concourse.bass_utils import run_bass_kernel_spmd

BF16 = ml_dtypes.bfloat16
P = 128
B, S, D = 4, 1024, 1280
KF = 4                  # frames folded per token
IN = D * KF             # 5120
H = 2048
O = 2048
E = 4
TOPK = 2
TK = B * (S // KF)      # 1024 tokens
KT = IN // P            # 40 contraction tiles
KG = 8                  # k-tiles per DMA group
NKG = KT // KG          # 5 groups
H1E = H // 2            # expert H half per core
ME = H1E // P           # 8
H1S = H // 8            # shared H slice per core
MS = H1S // P           # 2
NO = O // 512           # 4 output col tiles (shared mm2)
NOT = O // P            # 16 output col tiles (expert mm2)
OG = 4                  # expert-mm2 o-tiles per output DMA group
EPS_RMS = 1e-8
EPS_W = 1e-6
NCORES = 8


def host_preprocess(x, conv_w, conv_b, rms_w, router_w):
    """conv + fold + rmsnorm + router; returns (n [TK, IN] f32, combine [TK, E] f32)."""
    xp = np.pad(x, ((0, 0), (1, 1), (0, 0)))
    w0 = conv_w[:, 0, 0]
    w1 = conv_w[:, 0, 1]
    w2 = conv_w[:, 0, 2]
    xc = xp[:, :-2, :] * w0 + xp[:, 1:-1, :] * w1 + xp[:, 2:, :] * w2
    xr = x + xc + conv_b

    flat = xr.reshape(B, S // KF, IN).reshape(-1, IN)

    ms = np.mean(flat * flat, axis=-1, keepdims=True, dtype=np.float32)
    n = (flat * (1.0 / np.sqrt(ms + EPS_RMS)) * rms_w).astype(np.float32)

    logits = n @ router_w.T
    probs = 1.0 / (1.0 + np.exp(-logits))
    order = np.argsort(-probs, axis=1, kind="stable")
    idx = order[:, :TOPK]
    scores = np.take_along_axis(probs, idx, axis=1)
    w = scores / (scores.sum(axis=1, keepdims=True) + EPS_W)
    combine = np.zeros((n.shape[0], E), np.float32)
    rows = np.arange(n.shape[0])
    for j in range(TOPK):
        combine[rows, idx[:, j]] = w[:, j]
    return n, combine


def build_nc(TE, cnt=None, reps=1):
    """One SPMD program for all 8 cores.

    TE: kept for signature compatibility (unused for layout now).
    cnt: actual max token count over experts.
    reps>1 wraps the body in a hardware loop (benchmark use only: repeats
    are idempotent; used for differential wall-clock timing).
    """
    if cnt is None:
        cnt = TE
    assert cnt <= TK
    cw = max(0, cnt - 512)   # expert chunk-B width (tokens 512..cnt)
    ca = min(cnt, 512)       # expert chunk-A width
    dt = mybir.dt
    nc = bacc.Bacc()

    ntok_d = nc.dram_tensor("ntok", [2, NKG, P, KG, 512], dt.bfloat16, kind="ExternalInput")
    ew1t_d = nc.dram_tensor("ew1t", [ME, P, KT, P], dt.bfloat16, kind="ExternalInput")
    ew2t_d = nc.dram_tensor("ew2t", [P, ME, O], dt.bfloat16, kind="ExternalInput")
    w1sh_d = nc.dram_tensor("w1sh", [P, KT, H1S], dt.bfloat16, kind="ExternalInput")
    w2sh_d = nc.dram_tensor("w2sh", [P, MS, O], dt.bfloat16, kind="ExternalInput")
    b1e_d = nc.dram_tensor("b1e", [P, ME], dt.float32, kind="ExternalInput")
    b1s_d = nc.dram_tensor("b1s", [P, MS], dt.float32, kind="ExternalInput")
    oute_d = nc.dram_tensor("oute", [NOT // OG, P, OG, cnt], dt.bfloat16, kind="ExternalOutput")
    outs_d = nc.dram_tensor("outs", [TK, O], dt.bfloat16, kind="ExternalOutput")

    relu = mybir.ActivationFunctionType.Relu

    with tile.TileContext(nc) as tc:
        with (
            tc.tile_pool(name="res", bufs=1) as res,
            tc.tile_pool(name="wp", bufs=3) as wp,
            tc.tile_pool(name="npl", bufs=2 * NKG) as npl,
            tc.tile_pool(name="opl", bufs=3) as opl,
            tc.tile_pool(name="psp", bufs=8, space="PSUM") as psp,
        ):

            def emit_body():
                w1sh = res.tile([P, KT, H1S], dt.bfloat16, name="w1sh")
                ew2t = res.tile([P, ME, O], dt.bfloat16, name="ew2t")
                w2sh = res.tile([P, MS, O], dt.bfloat16, name="w2sh")
                b1e = res.tile([P, ME], dt.float32, name="b1e")
                b1s = res.tile([P, MS], dt.float32, name="b1s")
                hte = res.tile([P, ME, cnt], dt.bfloat16, name="hte")
                hts = res.tile([P, MS, TK], dt.bfloat16, name="hts")

                # ---- DMA emission: two FIFO queues, each in consumption order.
                # sync (SP): ci=0 tokens + w1sh, then expert mm1 weights.
                # gpsimd (Pool): ci=1 tokens, biases, mm2 weights, oute writes.
                # Pilot-split the g=0 streams so PE starts within ~2us.
                nt = [[None] * NKG for _ in range(2)]
                for g in range(NKG):
                    for ci in range(2):
                        nt[ci][g] = npl.tile([P, KG, 512], dt.bfloat16, tag="ntok", name="nt")
                # sync: w1sh g0..2 + nt0 stream; gp: nt1 stream + w1sh g3..4.
                for lo, hi in ((0, 2), (2, 5), (5, KG)):
                    nc.sync.dma_start(w1sh[:, lo:hi], w1sh_d[:, lo:hi])
                    nc.sync.dma_start(nt[0][0][:, lo:hi], ntok_d[0, 0][:, lo:hi])
                    nc.gpsimd.dma_start(nt[1][0][:, lo:hi], ntok_d[1, 0][:, lo:hi])
                    if lo == 0:
                        nc.gpsimd.dma_start(b1s[:], b1s_d[:])
                for g in range(1, NKG):
                    if g < 3:
                        nc.sync.dma_start(
                            w1sh[:, g * KG : (g + 1) * KG], w1sh_d[:, g * KG : (g + 1) * KG]
                        )
                    nc.sync.dma_start(nt[0][g][:], ntok_d[0, g])
                    nc.gpsimd.dma_start(nt[1][g][:], ntok_d[1, g])
                    if g >= 3:
                        nc.gpsimd.dma_start(
                            w1sh[:, g * KG : (g + 1) * KG], w1sh_d[:, g * KG : (g + 1) * KG]
                        )
                nc.gpsimd.dma_start(b1e[:], b1e_d[:])
                nc.gpsimd.dma_start(ew2t[:], ew2t_d[:])
                nc.gpsimd.dma_start(w2sh[:], w2sh_d[:])
                # Expert-side loads: wp bufs=3 means wt[3:] gate on consumption;
                # nothing after them on the sync queue until phase D's writes.
                wts = []
                for m in range(ME):
                    wt = wp.tile([P, KT, P], dt.bfloat16, tag="w1e", name="wt")
                    nc.sync.dma_start(wt[:], ew1t_d[m])
                    wts.append(wt)

                # ---- phase A: shared mm1, both token halves per k-tile ----
                pss = [
                    psp.tile([P, 512], dt.float32, tag="ps", name="ps_s1")
                    for _ in range(2 * MS)
                ]
                for g in range(NKG):
                    for kk in range(KG):
                        k = g * KG + kk
                        for m in range(MS):
                            for ci in range(2):
                                nc.tensor.matmul(
                                    pss[2 * m + ci],
                                    w1sh[:, k, m * P : (m + 1) * P],
                                    nt[ci][g][:, kk],
                                    start=(k == 0),
                                    stop=(k == KT - 1),
                                )
                for m in range(MS):
                    for ci in range(2):
                        nc.scalar.activation(
                            hts[:, m, ci * 512 : (ci + 1) * 512],
                            pss[2 * m + ci],
                            relu,
                            bias=b1s[:, m : m + 1],
                            scale=1.0,
                        )

                # ---- phase B: expert mm1 from resident token tiles ----
                for m in range(ME):
                    wt = wts[m]
                    psa = psp.tile([P, 512], dt.float32, tag="ps", name="ps_e1")[:, :ca]
                    psb = None
                    if cw:
                        psb = psp.tile([P, 512], dt.float32, tag="ps", name="ps_e1b")[:, :cw]
                    for g in range(NKG):
                        for kk in range(KG):
                            k = g * KG + kk
                            nc.tensor.matmul(
                                psa,
                                wt[:, k],
                                nt[0][g][:, kk, :ca],
                                start=(k == 0),
                                stop=(k == KT - 1),
                            )
                            if cw:
                                nc.tensor.matmul(
                                    psb,
                                    wt[:, k],
                                    nt[1][g][:, kk, :cw],
                                    start=(k == 0),
                                    stop=(k == KT - 1),
                                )
                    nc.scalar.activation(
                        hte[:, m, 0:ca], psa, relu, bias=b1e[:, m : m + 1], scale=1.0
                    )
                    if cw:
                        nc.scalar.activation(
                            hte[:, m, 512:cnt], psb, relu, bias=b1e[:, m : m + 1], scale=1.0
                        )

                # ---- phase D: shared mm2: outs rows = hts.T @ w2sh ----
                # (before expert mm2 so the heavy outs writes drain mid-kernel;
                # all writes ride the scalar queue so sync/gpsimd stay pure
                # input streams and the next rep's loads never sit behind them)
                for t in range(TK // P):
                    pso = [
                        psp.tile([P, 512], dt.float32, tag="ps", name="ps_o")
                        for _ in range(NO)
                    ]
                    for k in range(MS):
                        for o in range(NO):
                            nc.tensor.matmul(
                                pso[o],
                                hts[:, k, t * P : (t + 1) * P],
                                w2sh[:, k, o * 512 : (o + 1) * 512],
                                start=(k == 0),
                                stop=(k == MS - 1),
                            )
                    ot = opl.tile([P, O], dt.bfloat16, tag="outs", name="ot_s")
                    for o in range(NO):
                        if o % 2 == 0:
                            nc.vector.tensor_copy(ot[:, o * 512 : (o + 1) * 512], pso[o])
                        else:
                            nc.scalar.copy(ot[:, o * 512 : (o + 1) * 512], pso[o])
                    # split writes between the idle sync queue and scalar; the
                    # next rep's sync loads still clear ~6us before they gate
                    if t % 2 == 0:
                        nc.sync.dma_start(outs_d[t * P : (t + 1) * P], ot[:])
                    else:
                        nc.scalar.dma_start(outs_d[t * P : (t + 1) * P], ot[:])

                # ---- phase C: expert mm2, O on partitions, exact cnt columns ----
                for og in range(NOT // OG):
                    ob = opl.tile([P, OG, cnt], dt.bfloat16, tag="oute", name="ob")
                    for j in range(OG):
                        o = og * OG + j
                        psa = psp.tile([P, 512], dt.float32, tag="ps", name="ps_o2")[:, :ca]
                        psb = None
                        if cw:
                            psb = psp.tile([P, 512], dt.float32, tag="ps", name="ps_o2b")[:, :cw]
                        for k in range(ME):
                            nc.tensor.matmul(
                                psa,
                                ew2t[:, k, o * P : (o + 1) * P],
                                hte[:, k, 0:ca],
                                start=(k == 0),
                                stop=(k == ME - 1),
                            )
                            if cw:
                                nc.tensor.matmul(
                                    psb,
                                    ew2t[:, k, o * P : (o + 1) * P],
                                    hte[:, k, 512:cnt],
                                    start=(k == 0),
                                    stop=(k == ME - 1),
                                )
                        if j % 2 == 0:
                            nc.vector.tensor_copy(ob[:, j, 0:ca], psa)
                            if cw:
                                nc.vector.tensor_copy(ob[:, j, 512:cnt], psb)
                        else:
                            nc.scalar.copy(ob[:, j, 0:ca], psa)
                            if cw:
                                nc.scalar.copy(ob[:, j, 512:cnt], psb)
                        if og == NOT // OG - 1:
                            # last group: per-column-group writes shorten drain
                            nc.scalar.dma_start(oute_d[og, :, j], ob[:, j])
                    if og < NOT // OG - 1:
                        nc.scalar.dma_start(oute_d[og], ob[:])

            # reps>1 unrolls the body (no hardware loop: gpsimd-queue DMAs
            # did not re-arm reliably under tc.For_i on hardware).
            for _ in range(reps):
                emit_body()

    nc.finalize()
    return nc


def _prepare(inputs):
    inp = {k: np.asarray(v, dtype=np.float32) for k, v in inputs.items()}
    n, combine = host_preprocess(
        inp["x"], inp["conv_w"], inp["conv_b"], inp["rms_w"], inp["router_w"]
    )
    nbf = n.astype(BF16)

    idxs = [np.nonzero(combine[:, e] > 0)[0] for e in range(E)]
    maxcnt = max(1, max(len(ix) for ix in idxs))
    TE = int(math.ceil(maxcnt / P) * P)

    all_tokens = np.arange(TK)
    perms = []
    in_maps = []
    for c in range(NCORES):
        e, hh = divmod(c, 2)
        sl = slice(hh * H1E, (hh + 1) * H1E)
        # ew1t[m, p, k, q] = W1h[m*128+q, k*128+p]  (lhsT layout, contiguous per (m,p))
        W1h = inp["ew1"][e, sl]                      # [H1E, IN]
        ew1t = np.ascontiguousarray(
            W1h.reshape(ME, P, KT, P).transpose(0, 3, 2, 1)
        ).astype(BF16)
        W2h = inp["ew2"][e][:, sl]                   # [O, H1E]
        ew2t = np.ascontiguousarray(
            W2h.T.reshape(ME, P, O).transpose(1, 0, 2)
        ).astype(BF16)
        ssl = slice(c * H1S, (c + 1) * H1S)
        w1sh = np.ascontiguousarray(
            inp["sw1"][ssl].T.reshape(KT, P, H1S).transpose(1, 0, 2)
        ).astype(BF16)
        w2sh = np.ascontiguousarray(
            inp["sw2"][:, ssl].T.reshape(MS, P, O).transpose(1, 0, 2)
        ).astype(BF16)
        b1e = np.ascontiguousarray(inp["eb1"][e, sl].reshape(ME, P).T).astype(np.float32)
        b1s = np.ascontiguousarray(inp["sb1"][ssl].reshape(MS, P).T).astype(np.float32)

        idx_e = idxs[e]
        # permute tokens so this core's expert tokens come first; the expert
        # matmuls then read the prefix of the resident shared-expert tiles
        mask = np.zeros(TK, bool)
        mask[idx_e] = True
        perm = np.concatenate([idx_e, all_tokens[~mask]])
        perms.append(perm)
        ntok = np.ascontiguousarray(
            nbf[perm].T.reshape(NKG, KG, P, 2, 512).transpose(3, 0, 2, 1, 4)
        )

        in_maps.append(
            {
                "ntok": ntok,
                "ew1t": ew1t,
                "ew2t": ew2t,
                "w1sh": w1sh,
                "w2sh": w2sh,
                "b1e": b1e,
                "b1s": b1s,
            }
        )
    return inp, combine, idxs, perms, TE, in_maps


def _assemble(inp, combine, idxs, perms, results):
    acc = np.zeros((TK, O), np.float32)
    for c in range(NCORES):
        acc[perms[c]] += results[c]["outs"].astype(np.float32)
    acc += inp["sb2"][None, :]
    acc += combine @ inp["eb2"]
    for c in range(NCORES):
        e = c // 2
        idx_e = idxs[e]
        cnt = len(idx_e)
        if cnt:
            # oute [NOT//OG, P, OG, maxcnt]: O index = (og*OG + j)*P + p
            oe = results[c]["oute"].astype(np.float32)
            maxcnt = oe.shape[-1]
            oe = oe.transpose(3, 0, 2, 1).reshape(maxcnt, O)[:cnt]
            acc[idx_e] += oe * combine[idx_e, e][:, None]
    return acc.reshape(B, S // KF, O)


def run(inputs, trace=False):
    inp, combine, idxs, perms, TE, in_maps = _prepare(inputs)
    maxcnt = max(1, max(len(ix) for ix in idxs))
    nc = build_nc(TE, cnt=maxcnt)
    res = run_bass_kernel_spmd(nc, in_maps, core_ids=list(range(NCORES)), trace=trace)
    out = _assemble(inp, combine, idxs, perms, res.results)
    return out, res


def kernel(**inputs):
    out, _ = run(inputs, trace=False)
    return out
